# revision 15
# baseline (speedup 1.0000x reference)
"""TRN2 Bass kernel for sliding-window causal GQA attention block.

Reference computation (B=2, T=2048, C=2048, NH=16, NKV=4, HD=128, WIN=512):
  qkv = x @ w_qkv.T ; RoPE(q, k) ; GQA repeat ; banded causal attention
  (keys j in [i-511, i]) ; y @ w_proj.T

Sharding: 8 cores = batch (2) x kv-head-group (4) tensor parallel.
Core c = b*4+g owns batch b, q-heads [4g..4g+4), kv head g. Each core
computes a partial output (contribution of its 512 y-dims to all 2048
out dims); host sums the 4 partials per batch.

Everything on-chip is kept "transposed" ([feature, token]) so that all
matmuls have their contraction on the partition axis without any
on-chip layout changes except small PE transposes for probs/v/y.
"""
import sys
sys.path.insert(0, '/opt/trn_rl_repo')
import numpy as np
import ml_dtypes

import concourse.bass as bass
from concourse import bacc
import concourse.tile as tile
from concourse import mybir
from concourse.bass_utils import run_bass_kernel_spmd
from concourse.masks import make_identity

T = 2048
C = 2048
HD = 128
NH = 16
NKV = 4
NQL = 4           # q heads per core
WIN = 512
QKVF = NQL * HD + 2 * HD   # 768 local qkv features
SCALE = float(1.0 / np.sqrt(HD))
QB = T // 128     # 16 q blocks
KC = C // 128     # 16 contraction tiles
NEG = -1e9

f32 = mybir.dt.float32
f32r = mybir.dt.float32r
bf16 = mybir.dt.bfloat16

_CACHE = {}


def _build_program():
    nc = bacc.Bacc()
    xT = nc.declare_dram_parameter("xT", [C, T], f32r, isOutput=False)
    wqkvT = nc.declare_dram_parameter("wqkvT", [C, QKVF], f32r, isOutput=False)
    wpT = nc.declare_dram_parameter("wpT", [NQL * HD, C], bf16, isOutput=False)
    cosE = nc.declare_dram_parameter("cosE", [HD, T], f32, isOutput=False)
    sinE = nc.declare_dram_parameter("sinE", [HD, T], f32, isOutput=False)
    mdiag = nc.declare_dram_parameter("mdiag", [128, 128], f32, isOutput=False)
    mwin = nc.declare_dram_parameter("mwin", [128, 128], f32, isOutput=False)
    rotP = nc.declare_dram_parameter("rotP", [128, 128], f32r, isOutput=False)
    outT = nc.declare_dram_parameter("outT", [C, T], f32, isOutput=True)

    with tile.TileContext(nc) as tc:
        with tc.tile_pool(name="persist", bufs=1) as persist, \
             tc.tile_pool(name="qkv_out", bufs=1) as qkv_out, \
             tc.tile_pool(name="ytile", bufs=1) as ytile, \
             tc.tile_pool(name="outst", bufs=3) as outst:

            # ---- persistent small tensors ----
            cos_sb = persist.tile([HD, T], f32, tag="cos")
            sin_sb = persist.tile([HD, T], f32, tag="sin")
            nc.sync.dma_start(out=cos_sb, in_=cosE[:])
            nc.sync.dma_start(out=sin_sb, in_=sinE[:])
            md_sb = persist.tile([128, 128], f32, tag="md")
            mw_sb = persist.tile([128, 128], f32, tag="mw")
            nc.sync.dma_start(out=md_sb, in_=mdiag[:])
            nc.sync.dma_start(out=mw_sb, in_=mwin[:])
            ident_b = persist.tile([128, 128], bf16, tag="idb")
            make_identity(nc, ident_b)
            rp_sb = persist.tile([128, 128], f32r, tag="rp")
            nc.sync.dma_start(out=rp_sb, in_=rotP[:])
            wp_sb = persist.tile([128, NQL, C], bf16, tag="wp")
            nc.sync.dma_start(out=wp_sb, in_=wpT[:].rearrange("(kd p) o -> p kd o", p=128))

            # qkv outputs (transposed layout [feature, token])
            qT = [qkv_out.tile([HD, T], f32r, tag=f"qT{h}", name=f"qT{h}")
                  for h in range(NQL)]
            kT = qkv_out.tile([HD, T], f32r, tag="kT")
            vTb = qkv_out.tile([HD, T], bf16, tag="vTb")
            # v in [token, d] layout (bf16), per 128-token tile
            v_sb = [qkv_out.tile([128, HD], bf16, tag=f"v{t}", name=f"v{t}")
                    for t in range(QB)]
            # normalized attention output, transposed [d, token], bf16
            yT = [ytile.tile([HD, T], bf16, tag=f"yT{h}", name=f"yT{h}")
                  for h in range(NQL)]

            # ---- phase 1: QKV projection (f32r) ----
            with tc.tile_pool(name="wq", bufs=1) as wqp, \
                 tc.tile_pool(name="xs", bufs=16) as xsp, \
                 tc.tile_pool(name="qkps", bufs=3, space="PSUM") as qkps:
                w_sb = wqp.tile([128, KC, QKVF], f32r, tag="w")
                nc.sync.dma_start(out=w_sb, in_=wqkvT[:].rearrange("(kt p) f -> p kt f", p=128))
                for n in range(T // 512):
                    xt = []
                    for k in range(KC):
                        x_nk = xsp.tile([128, 512], f32r, tag="x")
                        nc.sync.dma_start(out=x_nk, in_=xT[k * 128:(k + 1) * 128, n * 512:(n + 1) * 512])
                        xt.append(x_nk)
                    for m in range(QKVF // 128):
                        acc = qkps.tile([128, 512], f32, tag="acc")
                        for k in range(KC):
                            nc.tensor.matmul(acc, w_sb[:, k, m * 128:(m + 1) * 128],
                                             xt[k],
                                             start=(k == 0), stop=(k == KC - 1))
                        ns = slice(n * 512, (n + 1) * 512)
                        if m < NQL:
                            nc.scalar.copy(out=qT[m][:, ns], in_=acc)
                        elif m == NQL:
                            nc.scalar.copy(out=kT[:, ns], in_=acc)
                        else:
                            nc.scalar.copy(out=vTb[:, ns], in_=acc)

            # ---- phase 2: RoPE ----
            # rotate-pair is done on the PE with a signed permutation
            # matrix (rot = P.T @ src, rot[2r] = -src[2r+1],
            # rot[2r+1] = src[2r]); DVE can't cross partitions.
            with tc.tile_pool(name="rope_tmp", bufs=3) as rtp, \
                 tc.tile_pool(name="ropeps", bufs=3, space="PSUM") as rops:
                for th in range(NQL + 1):
                    src = qT[th] if th < NQL else kT
                    for ch in range(T // 512):
                        cs = slice(ch * 512, (ch + 1) * 512)
                        rot = rops.tile([HD, 512], f32, tag="rot")
                        nc.tensor.matmul(rot, rp_sb, src[:, cs], start=True, stop=True)
                        tmp = rtp.tile([HD, 512], f32, tag="tmp")
                        nc.vector.tensor_mul(out=tmp, in0=rot, in1=sin_sb[:, cs])
                        nc.vector.tensor_mul(out=src[:, cs], in0=src[:, cs], in1=cos_sb[:, cs])
                        nc.vector.tensor_add(out=src[:, cs], in0=src[:, cs], in1=tmp)

            # ---- phase 2b: V transpose to [token, d] (bf16) ----
            with tc.tile_pool(name="vtps", bufs=3, space="PSUM") as vtps:
                for t in range(QB):
                    vp = vtps.tile([128, HD], bf16, tag="vp")
                    nc.tensor.transpose(vp, vTb[:, t * 128:(t + 1) * 128], ident_b)
                    nc.vector.tensor_copy(out=v_sb[t], in_=vp)

            # ---- phase 3: banded attention ----
            with tc.tile_pool(name="attn_sb", bufs=4) as asb, \
                 tc.tile_pool(name="scps", bufs=2, space="PSUM") as scps, \
                 tc.tile_pool(name="ptps", bufs=2, space="PSUM") as ptps, \
                 tc.tile_pool(name="ypps", bufs=2, space="PSUM") as ypps:
                for h in range(NQL):
                    for qb in range(QB):
                        kt_lo = max(0, qb - 4)
                        nk = qb - kt_lo + 1
                        w = nk * 128
                        sc = scps.tile([128, 640], f32, tag="sc")
                        lhs_q = qT[h][:, qb * 128:(qb + 1) * 128]
                        for c0 in range(0, w, 512):
                            cw = min(512, w - c0)
                            nc.tensor.matmul(sc[:, c0:c0 + cw],
                                             lhs_q,
                                             kT[:, kt_lo * 128 + c0: kt_lo * 128 + c0 + cw],
                                             start=True, stop=True)
                        # masks: causal on the diagonal tile, window on the leftmost
                        nc.vector.tensor_add(out=sc[:, w - 128:w], in0=sc[:, w - 128:w], in1=md_sb)
                        if qb >= 4:
                            nc.vector.tensor_add(out=sc[:, 0:128], in0=sc[:, 0:128], in1=mw_sb)
                        probs = asb.tile([128, 640], bf16, tag="probs")
                        ssum = asb.tile([128, 1], f32, tag="ssum")
                        nc.scalar.activation(out=probs[:, :w], in_=sc[:, :w],
                                             func=mybir.ActivationFunctionType.Exp,
                                             scale=SCALE, accum_out=ssum)
                        yp = ypps.tile([128, HD], f32, tag="yp")
                        for j in range(nk):
                            pt = ptps.tile([128, 128], bf16, tag="pt")
                            nc.tensor.transpose(pt, probs[:, j * 128:(j + 1) * 128], ident_b)
                            pts = asb.tile([128, 128], bf16, tag="pts")
                            nc.vector.tensor_copy(out=pts, in_=pt)
                            nc.tensor.matmul(yp, pts, v_sb[kt_lo + j],
                                             start=(j == 0), stop=(j == nk - 1))
                        rr = asb.tile([128, 1], f32, tag="rr")
                        nc.vector.reciprocal(rr, ssum)
                        yb = asb.tile([128, HD], bf16, tag="yb")
                        nc.vector.tensor_scalar_mul(yb, yp, rr)
                        ytp = ptps.tile([128, 128], bf16, tag="pt")
                        nc.tensor.transpose(ytp, yb, ident_b)
                        nc.vector.tensor_copy(out=yT[h][:, qb * 128:(qb + 1) * 128], in_=ytp)

            # ---- phase 4: output projection (bf16) ----
            with tc.tile_pool(name="pjps", bufs=3, space="PSUM") as pjps:
                for mo in range(C // 128):
                    for n in range(T // 512):
                        pp = pjps.tile([128, 512], f32, tag="pp")
                        for kd in range(NQL):
                            nc.tensor.matmul(pp, wp_sb[:, kd, mo * 128:(mo + 1) * 128],
                                             yT[kd][:, n * 512:(n + 1) * 512],
                                             start=(kd == 0), stop=(kd == NQL - 1))
                        os_t = outst.tile([128, 512], f32, tag="os")
                        nc.scalar.copy(out=os_t, in_=pp)
                        nc.sync.dma_start(out=outT[mo * 128:(mo + 1) * 128, n * 512:(n + 1) * 512],
                                          in_=os_t)
    nc.finalize()
    return nc


def _prep_inputs(x, w_qkv, w_proj, freqs_cos, freqs_sin):
    """Build the 8 per-core input maps (host-side shard + transpose)."""
    x = np.asarray(x, dtype=np.float32)
    w_qkv = np.asarray(w_qkv, dtype=np.float32)
    w_proj = np.asarray(w_proj, dtype=np.float32)
    freqs_cos = np.asarray(freqs_cos, dtype=np.float32)
    freqs_sin = np.asarray(freqs_sin, dtype=np.float32)

    # interleaved-pair rope tables expanded to [HD, T]
    cosE = np.ascontiguousarray(np.repeat(freqs_cos.T, 2, axis=0))
    sinE = np.ascontiguousarray(np.repeat(freqs_sin.T, 2, axis=0))
    # signed pair-rotation matrix: rot = P.T @ t, rot[2r] = -t[2r+1], rot[2r+1] = t[2r]
    rotP = np.zeros((HD, HD), np.float32)
    idx = np.arange(0, HD, 2)
    rotP[idx + 1, idx] = -1.0
    rotP[idx, idx + 1] = 1.0
    r = np.arange(128)[:, None]
    jj = np.arange(128)[None, :]
    mdiag = np.where(jj <= r, 0.0, NEG).astype(np.float32)
    mwin = np.where(jj > r, 0.0, NEG).astype(np.float32)

    xTs = [np.ascontiguousarray(x[b].T) for b in range(2)]
    in_maps = []
    for c in range(8):
        b, g = divmod(c, 4)
        wq = w_qkv[g * NQL * HD:(g + 1) * NQL * HD]          # [512, C]
        wk = w_qkv[NH * HD + g * HD: NH * HD + (g + 1) * HD]  # [128, C]
        wv = w_qkv[(NH + NKV) * HD + g * HD: (NH + NKV) * HD + (g + 1) * HD]
        wqkvT = np.ascontiguousarray(np.concatenate([wq, wk, wv], axis=0).T)
        wpT = np.ascontiguousarray(
            w_proj[:, g * NQL * HD:(g + 1) * NQL * HD].T).astype(ml_dtypes.bfloat16)
        in_maps.append({
            "xT": xTs[b], "wqkvT": wqkvT, "wpT": wpT,
            "cosE": cosE, "sinE": sinE, "mdiag": mdiag, "mwin": mwin,
            "rotP": rotP,
        })
    return in_maps


def _run(in_maps, trace=False):
    if "nc" not in _CACHE:
        _CACHE["nc"] = _build_program()
    return run_bass_kernel_spmd(_CACHE["nc"], in_maps, core_ids=list(range(8)),
                                trace=False)


def kernel(x, w_qkv, w_proj, freqs_cos, freqs_sin, mask=None, _trace=False):
    in_maps = _prep_inputs(x, w_qkv, w_proj, freqs_cos, freqs_sin)
    res = _run(in_maps, trace=_trace)
    out = np.empty((2, T, C), dtype=np.float32)
    for b in range(2):
        acc = res.results[b * 4]["outT"].astype(np.float32)
        for g in range(1, 4):
            acc = acc + res.results[b * 4 + g]["outT"]
        out[b] = acc.T
    if _trace:
        return out, res
    return out


# revision 35
# speedup vs baseline: 1.2971x; 1.2971x over previous
"""TRN2 Bass kernel for sliding-window causal GQA attention block.

Reference computation (B=2, T=2048, C=2048, NH=16, NKV=4, HD=128, WIN=512):
  qkv = x @ w_qkv.T ; RoPE(q, k) ; GQA repeat ; banded causal attention
  (keys j in [i-511, i]) ; y @ w_proj.T

Sharding: 8 cores = batch (2) x kv-head-group (4) tensor parallel.
Core c = b*4+g owns batch b, q-heads [4g..4g+4), kv head g. Each core
computes a partial output (contribution of its 512 y-dims to all 2048
out dims); host sums the 4 partials per batch.

Everything on-chip is kept "transposed" ([feature, token]) so that all
matmuls have their contraction on the partition axis without any
on-chip layout changes except small PE transposes for probs/v/y.
"""
import sys
sys.path.insert(0, '/opt/trn_rl_repo')
import numpy as np
import ml_dtypes

import concourse.bass as bass
from concourse import bacc
import concourse.tile as tile
from concourse import mybir
from concourse.bass_utils import run_bass_kernel_spmd
from concourse.masks import make_identity

T = 2048
C = 2048
HD = 128
NH = 16
NKV = 4
NQL = 4           # q heads per core
WIN = 512
QKVF = NQL * HD + 2 * HD   # 768 local qkv features
SCALE = float(1.0 / np.sqrt(HD))
QB = T // 128     # 16 q blocks
KC = C // 128     # 16 contraction tiles
NEG = -1e9

f32 = mybir.dt.float32
f32r = mybir.dt.float32r
bf16 = mybir.dt.bfloat16

_CACHE = {}


def _build_program():
    nc = bacc.Bacc()
    xT = nc.declare_dram_parameter("xT", [C, T], f32r, isOutput=False)
    wqkvT = nc.declare_dram_parameter("wqkvT", [C, QKVF], f32r, isOutput=False)
    wpT = nc.declare_dram_parameter("wpT", [NQL * HD, C], bf16, isOutput=False)
    cosE = nc.declare_dram_parameter("cosE", [HD, T], f32, isOutput=False)
    sinE = nc.declare_dram_parameter("sinE", [HD, T], f32, isOutput=False)
    mdiag = nc.declare_dram_parameter("mdiag", [128, 128], f32, isOutput=False)
    mwin = nc.declare_dram_parameter("mwin", [128, 128], f32, isOutput=False)
    rotP = nc.declare_dram_parameter("rotP", [128, 128], f32r, isOutput=False)
    outT = nc.declare_dram_parameter("outT", [C, T], f32, isOutput=True)

    import os as _os
    _tsim = _os.environ.get("KERNEL_TRACE_SIM", "0") == "1"
    with tile.TileContext(nc, trace_sim=_tsim) as tc:
        with tc.tile_pool(name="persist", bufs=1) as persist, \
             tc.tile_pool(name="qkv_out", bufs=1) as qkv_out, \
             tc.tile_pool(name="ytile", bufs=1) as ytile, \
             tc.tile_pool(name="outst", bufs=3) as outst, \
             tc.tile_pool(name="wq", bufs=1) as wqp, \
             tc.tile_pool(name="xs", bufs=18) as xsp, \
             tc.tile_pool(name="rope_tmp", bufs=2) as rtp, \
             tc.tile_pool(name="attn_sb", bufs=6) as asb:

            # ---- persistent small tensors (ACT HWDGE ring: off the
            # SP ring that streams weights/activations) ----
            cos_sb = persist.tile([HD, T], f32, tag="cos")
            sin_sb = persist.tile([HD, T], f32, tag="sin")
            nc.scalar.dma_start(out=cos_sb, in_=cosE[:])
            nc.scalar.dma_start(out=sin_sb, in_=sinE[:])
            md_sb = persist.tile([128, 128], f32, tag="md")
            mw_sb = persist.tile([128, 128], f32, tag="mw")
            nc.scalar.dma_start(out=md_sb, in_=mdiag[:])
            nc.scalar.dma_start(out=mw_sb, in_=mwin[:])
            rp_sb = persist.tile([128, 128], f32r, tag="rp")
            nc.scalar.dma_start(out=rp_sb, in_=rotP[:])
            ident_b = persist.tile([128, 128], bf16, tag="idb")
            make_identity(nc, ident_b)
            wp_sb = persist.tile([128, NQL, C], bf16, tag="wp")
            nc.scalar.dma_start(out=wp_sb, in_=wpT[:].rearrange("(kd p) o -> p kd o", p=128))

            # qkv outputs (transposed layout [feature, token])
            qT = [qkv_out.tile([HD, T], f32r, tag=f"qT{h}", name=f"qT{h}")
                  for h in range(NQL)]
            kT = qkv_out.tile([HD, T], f32r, tag="kT")
            vTb = qkv_out.tile([HD, T], bf16, tag="vTb")
            v_sb = [qkv_out.tile([128, HD], bf16, tag=f"v{t}", name=f"v{t}")
                    for t in range(QB)]
            yT = [ytile.tile([HD, T], bf16, tag=f"yT{h}", name=f"yT{h}")
                  for h in range(NQL)]

            # interleaved weight + first-chunk activation loads: the first
            # accumulation is DMA-arrival-paced
            w_tiles = []
            xt0 = []
            for k in range(KC):
                w_k = wqp.tile([128, QKVF], f32r, tag=f"w{k}", name=f"w{k}")
                nc.sync.dma_start(out=w_k, in_=wqkvT[k * 128:(k + 1) * 128, :])
                w_tiles.append(w_k)
                x_0k = xsp.tile([128, 512], f32r, tag="x", name=f"x0{k}")
                nc.sync.dma_start(out=x_0k, in_=xT[k * 128:(k + 1) * 128, 0:512])
                xt0.append(x_0k)

            def attn_block(h, qb, scps, ypps, ptps):
                kt_lo = max(0, qb - 4)
                nk = qb - kt_lo + 1
                w = nk * 128
                sc = scps.tile([128, 640], f32, tag="sc", name=f"sc{h}_{qb}")
                lhs_q = qT[h][:, qb * 128:(qb + 1) * 128]
                for c0 in range(0, w, 512):
                    cw = min(512, w - c0)
                    nc.tensor.matmul(sc[:, c0:c0 + cw], lhs_q,
                                     kT[:, kt_lo * 128 + c0: kt_lo * 128 + c0 + cw],
                                     start=True, stop=True)
                # causal mask on the diagonal tile, window mask on the leftmost
                nc.vector.tensor_add(out=sc[:, w - 128:w], in0=sc[:, w - 128:w], in1=md_sb)
                if qb >= 4:
                    nc.vector.tensor_add(out=sc[:, 0:128], in0=sc[:, 0:128], in1=mw_sb)
                probs = asb.tile([128, 640], bf16, tag="probs", name=f"pr{h}_{qb}")
                ssum = asb.tile([128, 1], f32, tag="ssum", name=f"ss{h}_{qb}")
                nc.scalar.activation(out=probs[:, :w], in_=sc[:, :w],
                                     func=mybir.ActivationFunctionType.Exp,
                                     scale=SCALE, accum_out=ssum)
                yp = ypps.tile([128, HD], f32, tag="yp", name=f"yp{h}_{qb}")
                for j in range(nk):
                    pt = ptps.tile([128, 128], bf16, tag="pt", name=f"ptp{h}_{qb}_{j}")
                    nc.tensor.transpose(pt, probs[:, j * 128:(j + 1) * 128], ident_b)
                    pts = asb.tile([128, 128], bf16, tag="pts", name=f"pt{h}_{qb}_{j}")
                    if j % 2 == 0:
                        nc.vector.tensor_copy(out=pts, in_=pt)
                    else:
                        nc.scalar.copy(out=pts, in_=pt)
                    nc.tensor.matmul(yp, pts, v_sb[kt_lo + j],
                                     start=(j == 0), stop=(j == nk - 1))
                rr = asb.tile([128, 1], f32, tag="rr", name=f"rr{h}_{qb}")
                nc.vector.reciprocal(rr, ssum)
                yb = asb.tile([128, HD], bf16, tag="yb", name=f"yb{h}_{qb}")
                nc.vector.tensor_scalar_mul(yb, yp, rr)
                ytp = ptps.tile([128, 128], bf16, tag="pt", name=f"ytp{h}_{qb}")
                nc.tensor.transpose(ytp, yb, ident_b)
                nc.scalar.copy(out=yT[h][:, qb * 128:(qb + 1) * 128], in_=ytp)

            def proj_chunk(n, pjps):
                for mo in range(C // 128):
                    pp = pjps.tile([128, 512], f32, tag="pp", name=f"pp{mo}_{n}")
                    for kd in range(NQL):
                        nc.tensor.matmul(pp, wp_sb[:, kd, mo * 128:(mo + 1) * 128],
                                         yT[kd][:, n * 512:(n + 1) * 512],
                                         start=(kd == 0), stop=(kd == NQL - 1))
                    os_t = outst.tile([128, 512], f32, tag="os", name=f"os{mo}_{n}")
                    nc.vector.tensor_copy(out=os_t, in_=pp)
                    nc.sync.dma_start(out=outT[mo * 128:(mo + 1) * 128, n * 512:(n + 1) * 512],
                                      in_=os_t)

            # ---- phase 1: QKV + rope + v-transpose, chunk by chunk ----
            with tc.tile_pool(name="qkps", bufs=3, space="PSUM") as qkps, \
                 tc.tile_pool(name="ropeps", bufs=2, space="PSUM") as rops:
                for n in range(T // 512):
                    ns = slice(n * 512, (n + 1) * 512)
                    if n == 0:
                        xt = xt0
                    else:
                        xt = []
                        for k in range(KC):
                            x_nk = xsp.tile([128, 512], f32r, tag="x")
                            nc.sync.dma_start(out=x_nk, in_=xT[k * 128:(k + 1) * 128, n * 512:(n + 1) * 512])
                            xt.append(x_nk)
                    # qkv for this 512-token chunk
                    for m in range(QKVF // 128):
                        acc = qkps.tile([128, 512], f32, tag="acc", name=f"acc{n}_{m}")
                        for k in range(KC):
                            nc.tensor.matmul(acc, w_tiles[k][:, m * 128:(m + 1) * 128],
                                             xt[k],
                                             start=(k == 0), stop=(k == KC - 1))
                        if m < NQL:
                            nc.scalar.copy(out=qT[m][:, ns], in_=acc)
                        elif m == NQL:
                            nc.scalar.copy(out=kT[:, ns], in_=acc)
                        else:
                            nc.scalar.copy(out=vTb[:, ns], in_=acc)
                    # rope this chunk (PE rotate via signed permutation
                    # matrix; DVE cannot cross partitions)
                    for th in range(NQL + 1):
                        src = qT[th] if th < NQL else kT
                        rot = rops.tile([HD, 512], f32, tag="rot", name=f"rot{n}_{th}")
                        nc.tensor.matmul(rot, rp_sb, src[:, ns], start=True, stop=True)
                        tmp = rtp.tile([HD, 512], f32, tag="tmp")
                        nc.vector.tensor_mul(out=tmp, in0=rot, in1=sin_sb[:, ns])
                        nc.vector.tensor_mul(out=src[:, ns], in0=src[:, ns], in1=cos_sb[:, ns])
                        nc.vector.tensor_add(out=src[:, ns], in0=src[:, ns], in1=tmp)
                    # v transpose (xbar DMA transpose, ACT ring)
                    for t in range(4 * n, 4 * n + 4):
                        nc.scalar.dma_start_transpose(v_sb[t], vTb[:, t * 128:(t + 1) * 128])

            # ---- phase 2: attention (qb-major) + per-chunk projection ----
            with tc.tile_pool(name="scps", bufs=1, space="PSUM") as scps, \
                 tc.tile_pool(name="ptps", bufs=2, space="PSUM") as ptps, \
                 tc.tile_pool(name="ypps", bufs=2, space="PSUM") as ypps, \
                 tc.tile_pool(name="pjps", bufs=2, space="PSUM") as pjps:
                globals_ns = {}
                for n in range(T // 512):
                    for qb in range(4 * n, 4 * n + 4):
                        for h in range(NQL):
                            attn_block(h, qb, scps, ypps, ptps)
                    proj_chunk(n, pjps)
    nc.finalize()
    return nc


def _prep_inputs(x, w_qkv, w_proj, freqs_cos, freqs_sin):
    """Build the 8 per-core input maps (host-side shard + transpose)."""
    x = np.asarray(x, dtype=np.float32)
    w_qkv = np.asarray(w_qkv, dtype=np.float32)
    w_proj = np.asarray(w_proj, dtype=np.float32)
    freqs_cos = np.asarray(freqs_cos, dtype=np.float32)
    freqs_sin = np.asarray(freqs_sin, dtype=np.float32)

    # interleaved-pair rope tables expanded to [HD, T]
    cosE = np.ascontiguousarray(np.repeat(freqs_cos.T, 2, axis=0))
    sinE = np.ascontiguousarray(np.repeat(freqs_sin.T, 2, axis=0))
    # signed pair-rotation matrix: rot = P.T @ t, rot[2r] = -t[2r+1], rot[2r+1] = t[2r]
    rotP = np.zeros((HD, HD), np.float32)
    idx = np.arange(0, HD, 2)
    rotP[idx + 1, idx] = -1.0
    rotP[idx, idx + 1] = 1.0
    r = np.arange(128)[:, None]
    jj = np.arange(128)[None, :]
    mdiag = np.where(jj <= r, 0.0, NEG).astype(np.float32)
    mwin = np.where(jj > r, 0.0, NEG).astype(np.float32)

    xTs = [np.ascontiguousarray(x[b].T) for b in range(2)]
    in_maps = []
    for c in range(8):
        b, g = divmod(c, 4)
        wq = w_qkv[g * NQL * HD:(g + 1) * NQL * HD]          # [512, C]
        wk = w_qkv[NH * HD + g * HD: NH * HD + (g + 1) * HD]  # [128, C]
        wv = w_qkv[(NH + NKV) * HD + g * HD: (NH + NKV) * HD + (g + 1) * HD]
        wqkvT = np.ascontiguousarray(np.concatenate([wq, wk, wv], axis=0).T)
        wpT = np.ascontiguousarray(
            w_proj[:, g * NQL * HD:(g + 1) * NQL * HD].T).astype(ml_dtypes.bfloat16)
        in_maps.append({
            "xT": xTs[b], "wqkvT": wqkvT, "wpT": wpT,
            "cosE": cosE, "sinE": sinE, "mdiag": mdiag, "mwin": mwin,
            "rotP": rotP,
        })
    return in_maps


def _run(in_maps, trace=False):
    if "nc" not in _CACHE:
        _CACHE["nc"] = _build_program()
    return run_bass_kernel_spmd(_CACHE["nc"], in_maps, core_ids=list(range(8)),
                                trace=False)


def kernel(x, w_qkv, w_proj, freqs_cos, freqs_sin, mask=None, _trace=False):
    in_maps = _prep_inputs(x, w_qkv, w_proj, freqs_cos, freqs_sin)
    res = _run(in_maps, trace=_trace)
    out = np.empty((2, T, C), dtype=np.float32)
    for b in range(2):
        acc = res.results[b * 4]["outT"].astype(np.float32)
        for g in range(1, 4):
            acc = acc + res.results[b * 4 + g]["outT"]
        out[b] = acc.T
    if _trace:
        return out, res
    return out
